# revision 30
# baseline (speedup 1.0000x reference)
"""Causal single-head attention (B=4, T=2048, D=1024, fp32) on 8 TRN2 NeuronCores.

Sharding: 2 cores per batch. Within a pair, keys/values are split by
interleaved 128-token tiles (core parity p takes s-tiles t with t%2==p), which
makes the program perfectly uniform across cores (one SPMD program, per-core
differences live entirely in the input data): for every 512-wide query chunk
i, each core processes exactly 2i+2 local key tiles, with the causal boundary
applied through two per-core additive mask tiles. Each core computes an
unnormalized partial attention output plus softmax denominators for ALL
queries of its batch; the host merges the two partials per batch (add, then
divide) while unsharding.

This version (~143us HW exec, max core; fp32r baseline was ~196us) vs that
baseline:
- All matmul operands are bf16 (fp32 PSUM accumulation): halves every DMA
  byte and LDWEIGHTS cost (113ns vs 215ns fp32r). N=512 matmuls then run at
  the ~216ns issue floor (512 cols / 2.4GHz). Logit abs err ~0.01 ->
  end-to-end rel err ~4.5e-3, under the 2e-2 gate.
- All inputs are loaded up-front into pinned SBUF tiles (everything fits in
  bf16: ~122KB/partition of 208KB) as ~19 multi-k-tile dma triggers (512KB-
  1MB each via rearranged DRAM APs, so all 16 SDMA engines fan out per
  trigger), issued in consumption order alternating between the two HWDGE
  rings (SP/ACT). The baseline staged tiles lazily per phase: ~187 triggers
  at ~0.61us sequencer cost each backlogged the rings, and the PE stalled
  ~20us at phase boundaries (plus ~20us of HAM half-clock penalties from the
  >3.4us idle windows those stalls opened).
- The PE is kept busy through the startup DMA window: warmup matmuls on a
  DVE-memset tile from ~7.7us (preamble end), then phase B's first two
  chains run k-banded in load-arrival order with dependency-free fillers
  between bands (a core whose loads land late must not open a >3.4us HAM
  window).
- ps512 runs 3 PSUM banks (psS 1) so score-tile exp chains and V-projection
  copies never gate matmul issue; chunk 0's score tiles interleave into
  phase C to hide their serial mask-add + exp latency.
- Output partials are written in bf16 (262KB per query block), one SP-ring
  trigger per block mid-kernel (ACT-ring output triggers would queue ahead
  of the next chunk's Exp), osb 4-deep against output-DMA completion
  variance, and the final chunk's copies/DMAs quartered so the tail drain
  pipelines behind the last matmuls.

Softmax runs without max-subtraction: logits = scores/32 stay within ~+-8
for this input distribution, far from overflow in fp32 PSUM / bf16 exp.

Measured phase map (NTFF, per core): preamble ~7.3us; matmul span ~8->135us
at ~216ns/MM median with <2us total gaps; outputs drain ~2us; teardown ~5us.
Matmul count: 16 warm + 4 fillers + 256 projection + 160 scores + 144 attn@V
+ 72 denominator (N=2, ride in ~40ns issue slots).
"""
import numpy as np

B, T, D = 4, 2048, 1024
P = 128
NK = D // P          # 8 contraction tiles
QC = T // 512        # 4 query chunks of 512
NEG = -1e30
SCALE = 1.0 / 32.0   # 1/sqrt(D)
N_WARM = 16

_prog = None
_last_in_maps = None


def _build_program():
    import concourse.bacc as bacc
    import concourse.mybir as mybir
    import concourse.tile as tile

    f32 = mybir.dt.float32
    bf = mybir.dt.bfloat16

    nc = bacc.Bacc()
    xt_d = nc.declare_dram_parameter("xt", [D, T], bf, isOutput=False)
    xtl_d = nc.declare_dram_parameter("xtl", [D, T // 2], bf, isOutput=False)
    wkq_d = nc.declare_dram_parameter("wkq", [D, D], bf, isOutput=False)
    wv_d = nc.declare_dram_parameter("wv", [D, D], bf, isOutput=False)
    mask_d = nc.declare_dram_parameter("masks", [2, P, 512], bf, isOutput=False)
    ones_d = nc.declare_dram_parameter("ones", [P, 2], bf, isOutput=False)
    part_d = nc.declare_dram_parameter("part", [T, D + 1], bf, isOutput=True)

    with tile.TileContext(nc) as tc:
        with tc.tile_pool(name="sbuf", bufs=1) as pool, \
             tc.tile_pool(name="psum", bufs=1, space="PSUM") as psum:

            # Alternate dma issue between the two HWDGE rings (SP/ACT):
            # each trigger costs ~0.61us on its issuing sequencer.
            _eng = [0]

            def dma(dst, src_ap):
                e = nc.sync if _eng[0] % 2 == 0 else nc.scalar
                _eng[0] += 1
                e.dma_start(dst, src_ap)

            # ---- pinned input/working tiles (all bf16) ----
            wq_sb = pool.tile([P, NK, D], bf, tag="wq")        # Wk@Wq^T rows
            xl_sb = pool.tile([P, NK, T // 2], bf, tag="xl")   # local s cols of x^T
            wv_sb = pool.tile([P, NK, D], bf, tag="wv")
            xt_sb = pool.tile([P, NK, T], bf, tag="xt")        # all queries, natural
            kt_sb = pool.tile([P, NK, T // 2], bf, tag="kt")   # K'^T, local s
            v_sb = pool.tile([P, NK, D], bf, tag="v")          # V, local s tiles
            mask_t = pool.tile([P, 2, 512], bf, tag="mask")
            ones_t = pool.tile([P, 2], bf, tag="ones")

            # ---- HAM pre-warm ----
            # PE sits behind the DMA load window at kernel start; throwaway
            # matmuls on a DVE-memset tile hold the clock gate at 8/8.
            # The warmup must cover until the critical-load k-chains are fed
            # (~20us): the first k-pair chains trickle in between warm MMs
            # ending and full feed, and any >3.4us idle window re-throttles
            # the PE clock to 1.2GHz for ~7us.
            warm = pool.tile([P, 512], bf, tag="warm")
            nc.vector.memset(warm[:], 0.0)
            wps = psum.tile([P, 512], f32, tag="ps512", bufs=3)
            for w in range(N_WARM):
                nc.tensor.matmul(wps[:], warm[:, 0:P], warm[:],
                                 start=(w == 0), stop=(w == N_WARM - 1))

            # ---- up-front loads, in consumption order ----
            # Multi-k-tile triggers: one dma_start covers [P, ks, cols] via a
            # rearranged DRAM AP, so all 16 SDMA engines fan out immediately
            # (per-k 128KB triggers fed the first chain only by ~17us; these
            # land it ~15us with 15 fewer issue slots). Phase B's first chain
            # (h=0, j=0) needs the h0 half-rows of wkq + j0 half-rows of xtl:
            # that critical 2MB goes first, split k0-3/k4-7 for pipelining,
            # wkq on SP and xtl on ACT.
            def big(dst_tile, src, k0, k1, c0, c1):
                dma(dst_tile[:, k0:k1, c0:c1],
                    src[k0 * P:k1 * P, c0:c1].rearrange("(k p) c -> p k c", p=P))

            big(wq_sb, wkq_d, 0, 4, 0, 512)
            big(xl_sb, xtl_d, 0, 4, 0, 512)
            big(wq_sb, wkq_d, 4, 6, 0, 512)
            big(xl_sb, xtl_d, 4, 6, 0, 512)
            big(wq_sb, wkq_d, 6, 8, 0, 512)
            big(xl_sb, xtl_d, 6, 8, 0, 512)
            big(xl_sb, xtl_d, 0, 4, 512, 1024)   # phase B j=1
            big(xl_sb, xtl_d, 4, 8, 512, 1024)
            big(wq_sb, wkq_d, 0, 4, 512, 1024)   # phase B h=1
            big(wq_sb, wkq_d, 4, 8, 512, 1024)
            big(wv_sb, wv_d, 0, 8, 0, 512)       # phase C n=0
            big(wv_sb, wv_d, 0, 8, 512, 1024)    # phase C n=1
            dma(mask_t[:, 0, :], mask_d[0])
            dma(mask_t[:, 1, :], mask_d[1])
            dma(ones_t[:], ones_d[:])
            for i in range(QC):                  # phase D chunk i
                big(xt_sb, xt_d, 0, 8, 512 * i, 512 * (i + 1))

            # ---- phase B: K'^T = (Wk Wq^T)^T x^T over local s ----
            # The first two chains run k-banded in the order the staged
            # critical loads land (k0-3 ~11.5us, k4-5 ~13.3, k6-7 ~15.1), so
            # the PE does real work during the load window instead of longer
            # warmup. Interleaved open accumulation groups across the two
            # ps512 banks are fine (same pattern as attn@V's pso/pss chains).
            psA = [psum.tile([P, 512], f32, tag="ps512", bufs=3, name=f"psA{_m}")
                   for _m in range(2)]
            for kb0, kb1 in ((0, 4), (4, 6), (6, 8)):
                for mm in range(2):
                    for k in range(kb0, kb1):
                        nc.tensor.matmul(psA[mm][:], wq_sb[:, k, mm * P:(mm + 1) * P],
                                         xl_sb[:, k, 0:512],
                                         start=(k == 0), stop=(k == NK - 1))
                if kb1 < NK:
                    # dependency-free fillers into the (dead) warm slot: on a
                    # core whose next k-band lands late (per-core DMA variance
                    # is ~1-2us), these keep the PE active so the >3.4us HAM
                    # re-throttle window never opens. Reuse wps directly -- a
                    # fresh ps512 allocation here would rotate onto a slot
                    # holding an open psA accumulation and its start=True
                    # would clear that bank.
                    for _f in range(2):
                        nc.tensor.matmul(wps[:], warm[:, 0:P], warm[:],
                                         start=True, stop=True)
            for mm in range(2):
                nc.vector.tensor_copy(kt_sb[:, mm, 0:512], psA[mm][:])
            for h in range(2):                     # dout halves
                for j in range(2):                 # local s 512-chunks
                    for mm in range(4):
                        if h == 0 and j == 0 and mm < 2:
                            continue
                        if h == 0 and j == 1 and mm == 0:
                            # same insurance at the j1-load boundary
                            for _f in range(2):
                                nc.tensor.matmul(wps[:], warm[:, 0:P], warm[:],
                                                 start=True, stop=True)
                        m = 4 * h + mm
                        c = 512 * h + mm * P
                        ps = psum.tile([P, 512], f32, tag="ps512", bufs=3)
                        for k in range(NK):
                            nc.tensor.matmul(ps[:], wq_sb[:, k, c:c + P],
                                             xl_sb[:, k, 512 * j:512 * (j + 1)],
                                             start=(k == 0), stop=(k == NK - 1))
                        nc.vector.tensor_copy(kt_sb[:, m, 512 * j:512 * (j + 1)], ps[:])

            # ---- phase D score helper ----
            # scores fold the Q projection into the host-precomputed wkq, so
            # the S^T matmul consumes resident x^T columns.
            def scores_tile(pt, i, lt):
                # the last local tile (lt == 2i+1) is fully masked for the
                # first 256 query columns AND excluded from their attn@V
                # accumulation (nlt), so only its right half is computed
                lo = 256 if lt == 2 * i + 1 else 0
                ps = psum.tile([P, 512 - lo], f32, tag="ps512", bufs=3)
                for m in range(NK):
                    nc.tensor.matmul(ps[:], kt_sb[:, m, lt * P:(lt + 1) * P],
                                     xt_sb[:, m, 512 * i + lo:512 * (i + 1)],
                                     start=(m == 0), stop=(m == NK - 1))
                if lt == 2 * i:
                    nc.vector.tensor_add(ps[:], ps[:], mask_t[:, 0, :])
                elif lt == 2 * i + 1:
                    nc.vector.tensor_add(ps[:], ps[:], mask_t[:, 1, 256:512])
                nc.scalar.activation(pt[:, lt, lo:512], ps[:],
                                     mybir.ActivationFunctionType.Exp,
                                     bias=0.0, scale=SCALE)

            def scores(i):
                pt = pool.tile([P, NK, 512], bf, tag="pt", bufs=2)
                for lt in range(2 * i + 2):
                    scores_tile(pt, i, lt)
                return pt

            # ---- phase C: V over local s ----
            def cgroup(n, j, lt4):
                lt = 4 * j + lt4
                ps = psum.tile([P, 512], f32, tag="ps512", bufs=3)
                for k in range(NK):
                    nc.tensor.matmul(ps[:],
                                     xl_sb[:, k, lt * P:(lt + 1) * P],
                                     wv_sb[:, k, 512 * n:512 * (n + 1)],
                                     start=(k == 0), stop=(k == NK - 1))
                nc.vector.tensor_copy(v_sb[:, lt, 512 * n:512 * (n + 1)], ps[:])

            # chunk 0's two score tiles are interleaved into phase C so each
            # serial mask-add + exp chain hides under a C group (exposed 0.7us
            # on the PE at any phase boundary otherwise), with two C groups
            # between the score tile and the next reuse of its PSUM slot.
            for lt4 in range(4):
                cgroup(0, 0, lt4)
            pt0 = pool.tile([P, NK, 512], bf, tag="pt", bufs=2)
            scores_tile(pt0, 0, 0)
            cgroup(0, 1, 0)
            cgroup(0, 1, 1)
            scores_tile(pt0, 0, 1)
            cgroup(0, 1, 2)
            cgroup(0, 1, 3)
            for n, j in ((1, 0), (1, 1)):
                for lt4 in range(4):
                    cgroup(n, j, lt4)

            # ---- phase D: per query chunk ----
            for i in range(QC):
                pt = pt0 if i == 0 else scores(i)
                qb_order = [3, 2, 1, 0] if i == QC - 1 else [0, 1, 2, 3]
                for qb in qb_order:
                    nlt = 2 * i + 1 if qb < 2 else 2 * i + 2
                    pso = psum.tile([P, D], f32, tag="psO", bufs=2)
                    pss = psum.tile([P, 2], f32, tag="psS", bufs=1)
                    for t_ in range(nlt):
                        lhs = pt[:, t_, qb * P:(qb + 1) * P]
                        st, sp = (t_ == 0), (t_ == nlt - 1)
                        nc.tensor.matmul(pso[:, 0:512], lhs, v_sb[:, t_, 0:512],
                                         start=st, stop=sp)
                        nc.tensor.matmul(pso[:, 512:1024], lhs, v_sb[:, t_, 512:1024],
                                         start=st, stop=sp)
                        nc.tensor.matmul(pss[:], lhs, ones_t[:], start=st, stop=sp)
                    # 4 bufs: output-DMA completion can take ~3-6us under
                    # cross-core HBM contention; 2 bufs convoys the casts
                    # (cast qb_n+2 gated on dma qb_n) and stalls psO recycling
                    osb = pool.tile([P, D + 1], bf, tag="osb", bufs=4)
                    r0 = 512 * i + qb * P
                    # output triggers go on SP only: on the ACT ring they
                    # queue ahead of the next chunk's Exp activations (FIFO)
                    # and stall PSUM recycling for ~1us per chunk boundary
                    if i < QC - 1:
                        # 262KB bf16 per block: one trigger mid-kernel
                        nc.vector.tensor_copy(osb[:, 0:D], pso[:])
                        nc.vector.tensor_copy(osb[:, D:D + 1], pss[:, 0:1])
                        nc.sync.dma_start(part_d[r0:r0 + P, :], osb[:])
                    else:
                        # tail: pipeline copy quarters with their DMAs so the
                        # drain starts ~400ns after the last matmul
                        nc.vector.tensor_copy(osb[:, D:D + 1], pss[:, 0:1])
                        # the tail is SP-ISSUE-bound: exec ends at the last
                        # output DMA's completion, and each trigger costs
                        # ~0.6us of serial sequencer time. In the final chunk
                        # ACT is free (all its exps precede attn@V), so
                        # alternate rings: the issue train shrinks from ~16
                        # SP triggers (~10us past the last matmul) to ~5 per
                        # ring. The very last block goes as 4 smaller pieces
                        # so its drain starts on the first PSUM bank's stop
                        # and the final transfer is ~66KB.
                        if qb == qb_order[-1]:
                            pieces = ((0, 256), (256, 512),
                                      (512, 768), (768, D + 1))
                        else:
                            pieces = ((0, 512), (512, D + 1))
                        for piece, (c_lo, c_hi) in enumerate(pieces):
                            nc.vector.tensor_copy(osb[:, c_lo:min(c_hi, D)],
                                                  pso[:, c_lo:min(c_hi, D)])
                            e = nc.sync if piece % 2 == 0 else nc.scalar
                            e.dma_start(part_d[r0:r0 + P, c_lo:c_hi],
                                        osb[:, c_lo:c_hi])

    nc.finalize()
    return nc


def _get_program():
    global _prog
    if _prog is None:
        _prog = _build_program()
    return _prog


def kernel(x, Wq, Wk, Wv):
    import ml_dtypes
    from concourse.bass_utils import run_bass_kernel_spmd

    bf16 = ml_dtypes.bfloat16
    x = np.asarray(x, dtype=np.float32)
    Wq = np.ascontiguousarray(np.asarray(Wq, dtype=np.float32))
    Wk = np.ascontiguousarray(np.asarray(Wk, dtype=np.float32))
    Wv = np.ascontiguousarray(np.asarray(Wv, dtype=np.float32))

    ones = np.ones((P, 2), dtype=bf16)
    # scores = x (Wq Wk^T) x^T: fold the two projection matrices on the host.
    # The device tensor plays the old Wk role: lhsT[b, a] = (Wk Wq^T)[b, a].
    Wkq = np.ascontiguousarray(
        (Wk.astype(np.float64) @ Wq.T.astype(np.float64)).astype(np.float32)
    ).astype(bf16)
    Wv_b = np.ascontiguousarray(Wv).astype(bf16)
    sr = np.arange(P)[:, None]
    qr = np.arange(512)[None, :]
    masks = {}
    for p in (0, 1):
        m0 = np.where(128 * p + sr > qr, NEG, 0.0).astype(bf16)
        m1 = np.where(128 * (2 + p) + sr > qr, NEG, 0.0).astype(bf16)
        masks[p] = np.stack([m0, m1])

    in_maps = []
    for c in range(8):
        b, p = c // 2, c % 2
        xt = np.ascontiguousarray(x[b].T)                     # [D, T]
        xtv = xt.reshape(D, T // P, P)
        xtl = np.ascontiguousarray(
            xtv[:, p::2, :].reshape(D, T // 2)).astype(bf16)  # local s cols
        in_maps.append({
            "xt": xt.astype(bf16), "xtl": xtl,
            "wkq": Wkq, "wv": Wv_b,
            "masks": masks[p], "ones": ones,
        })

    global _last_in_maps
    _last_in_maps = in_maps
    nc = _get_program()
    res = run_bass_kernel_spmd(nc, in_maps, list(range(8)))

    out = np.empty((B, T, D), dtype=np.float32)
    for b in range(B):
        p0 = res.results[2 * b]["part"].astype(np.float32)
        p1 = res.results[2 * b + 1]["part"].astype(np.float32)
        O = p0[:, :D] + p1[:, :D]
        d = p0[:, D] + p1[:, D]
        out[b] = O / d[:, None]
    return out


# revision 31
# speedup vs baseline: 1.0037x; 1.0037x over previous
"""Causal single-head attention (B=4, T=2048, D=1024, fp32) on 8 TRN2 NeuronCores.

Sharding: 2 cores per batch. Within a pair, keys/values are split by
interleaved 128-token tiles (core parity p takes s-tiles t with t%2==p), which
makes the program perfectly uniform across cores (one SPMD program, per-core
differences live entirely in the input data): for every 512-wide query chunk
i, each core processes exactly 2i+2 local key tiles, with the causal boundary
applied through two per-core additive mask tiles. Each core computes an
unnormalized partial attention output plus softmax denominators for ALL
queries of its batch; the host merges the two partials per batch (add, then
divide) while unsharding.

This version (~143us HW exec, max core; fp32r baseline was ~196us) vs that
baseline:
- All matmul operands are bf16 (fp32 PSUM accumulation): halves every DMA
  byte and LDWEIGHTS cost (113ns vs 215ns fp32r). N=512 matmuls then run at
  the ~216ns issue floor (512 cols / 2.4GHz). Logit abs err ~0.01 ->
  end-to-end rel err ~4.5e-3, under the 2e-2 gate.
- All inputs are loaded up-front into pinned SBUF tiles (everything fits in
  bf16: ~122KB/partition of 208KB) as ~19 multi-k-tile dma triggers (512KB-
  1MB each via rearranged DRAM APs, so all 16 SDMA engines fan out per
  trigger), issued in consumption order alternating between the two HWDGE
  rings (SP/ACT). The baseline staged tiles lazily per phase: ~187 triggers
  at ~0.61us sequencer cost each backlogged the rings, and the PE stalled
  ~20us at phase boundaries (plus ~20us of HAM half-clock penalties from the
  >3.4us idle windows those stalls opened).
- The PE is kept busy through the startup DMA window: warmup matmuls on a
  DVE-memset tile from ~7.7us (preamble end), then phase B's first two
  chains run k-banded in load-arrival order with dependency-free fillers
  between bands (a core whose loads land late must not open a >3.4us HAM
  window).
- ps512 runs 3 PSUM banks (psS 1) so score-tile exp chains and V-projection
  copies never gate matmul issue; chunk 0's score tiles interleave into
  phase C to hide their serial mask-add + exp latency.
- Output partials are written in bf16 (262KB per query block), one SP-ring
  trigger per block mid-kernel (ACT-ring output triggers would queue ahead
  of the next chunk's Exp), osb 4-deep against output-DMA completion
  variance, and the final chunk's copies/DMAs quartered so the tail drain
  pipelines behind the last matmuls.

Softmax runs without max-subtraction: logits = scores/32 stay within ~+-8
for this input distribution, far from overflow in fp32 PSUM / bf16 exp.

Measured phase map (NTFF, per core): preamble ~7.3us; matmul span ~8->135us
at ~216ns/MM median with <2us total gaps; outputs drain ~2us; teardown ~5us.
Matmul count: 16 warm + 4 fillers + 256 projection + 160 scores + 144 attn@V
+ 72 denominator (N=2, ride in ~40ns issue slots).
"""
import numpy as np

B, T, D = 4, 2048, 1024
P = 128
NK = D // P          # 8 contraction tiles
QC = T // 512        # 4 query chunks of 512
NEG = -1e30
SCALE = 1.0 / 32.0   # 1/sqrt(D)
N_WARM = 16

_prog = None
_last_in_maps = None


def _build_program():
    import concourse.bacc as bacc
    import concourse.mybir as mybir
    import concourse.tile as tile

    f32 = mybir.dt.float32
    bf = mybir.dt.bfloat16

    nc = bacc.Bacc()
    xt_d = nc.declare_dram_parameter("xt", [D, T], bf, isOutput=False)
    xtl_d = nc.declare_dram_parameter("xtl", [D, T // 2], bf, isOutput=False)
    wkq_d = nc.declare_dram_parameter("wkq", [D, D], bf, isOutput=False)
    wv_d = nc.declare_dram_parameter("wv", [D, D], bf, isOutput=False)
    mask_d = nc.declare_dram_parameter("masks", [2, P, 512], bf, isOutput=False)
    ones_d = nc.declare_dram_parameter("ones", [P, 2], bf, isOutput=False)
    part_d = nc.declare_dram_parameter("part", [T, D + 1], bf, isOutput=True)

    with tile.TileContext(nc) as tc:
        with tc.tile_pool(name="sbuf", bufs=1) as pool, \
             tc.tile_pool(name="psum", bufs=1, space="PSUM") as psum:

            # Alternate dma issue between the two HWDGE rings (SP/ACT):
            # each trigger costs ~0.61us on its issuing sequencer.
            _eng = [0]

            def dma(dst, src_ap):
                e = nc.sync if _eng[0] % 2 == 0 else nc.scalar
                _eng[0] += 1
                e.dma_start(dst, src_ap)

            # ---- pinned input/working tiles (all bf16) ----
            wq_sb = pool.tile([P, NK, D], bf, tag="wq")        # Wk@Wq^T rows
            xl_sb = pool.tile([P, NK, T // 2], bf, tag="xl")   # local s cols of x^T
            wv_sb = pool.tile([P, NK, D], bf, tag="wv")
            xt_sb = pool.tile([P, NK, T], bf, tag="xt")        # all queries, natural
            kt_sb = pool.tile([P, NK, T // 2], bf, tag="kt")   # K'^T, local s
            v_sb = pool.tile([P, NK, D], bf, tag="v")          # V, local s tiles
            mask_t = pool.tile([P, 2, 512], bf, tag="mask")
            ones_t = pool.tile([P, 2], bf, tag="ones")

            # ---- HAM pre-warm ----
            # PE sits behind the DMA load window at kernel start; throwaway
            # matmuls on a DVE-memset tile hold the clock gate at 8/8.
            # The warmup must cover until the critical-load k-chains are fed
            # (~20us): the first k-pair chains trickle in between warm MMs
            # ending and full feed, and any >3.4us idle window re-throttles
            # the PE clock to 1.2GHz for ~7us.
            warm = pool.tile([P, 512], bf, tag="warm")
            nc.vector.memset(warm[:], 0.0)
            wps = psum.tile([P, 512], f32, tag="ps512", bufs=3)
            for w in range(N_WARM):
                nc.tensor.matmul(wps[:], warm[:, 0:P], warm[:],
                                 start=(w == 0), stop=(w == N_WARM - 1))

            # ---- up-front loads, in consumption order ----
            # Multi-k-tile triggers: one dma_start covers [P, ks, cols] via a
            # rearranged DRAM AP, so all 16 SDMA engines fan out immediately
            # (per-k 128KB triggers fed the first chain only by ~17us; these
            # land it ~15us with 15 fewer issue slots). Phase B's first chain
            # (h=0, j=0) needs the h0 half-rows of wkq + j0 half-rows of xtl:
            # that critical 2MB goes first, split k0-3/k4-7 for pipelining,
            # wkq on SP and xtl on ACT.
            def big(dst_tile, src, k0, k1, c0, c1):
                dma(dst_tile[:, k0:k1, c0:c1],
                    src[k0 * P:k1 * P, c0:c1].rearrange("(k p) c -> p k c", p=P))

            big(wq_sb, wkq_d, 0, 4, 0, 512)
            big(xl_sb, xtl_d, 0, 4, 0, 512)
            big(wq_sb, wkq_d, 4, 6, 0, 512)
            big(xl_sb, xtl_d, 4, 6, 0, 512)
            big(wq_sb, wkq_d, 6, 8, 0, 512)
            big(xl_sb, xtl_d, 6, 8, 0, 512)
            big(xl_sb, xtl_d, 0, 4, 512, 1024)   # phase B j=1
            big(xl_sb, xtl_d, 4, 8, 512, 1024)
            big(wq_sb, wkq_d, 0, 4, 512, 1024)   # phase B h=1
            big(wq_sb, wkq_d, 4, 8, 512, 1024)
            big(wv_sb, wv_d, 0, 8, 0, 512)       # phase C n=0
            big(wv_sb, wv_d, 0, 8, 512, 1024)    # phase C n=1
            dma(mask_t[:, 0, :], mask_d[0])
            dma(mask_t[:, 1, :], mask_d[1])
            dma(ones_t[:], ones_d[:])
            for i in range(QC):                  # phase D chunk i
                big(xt_sb, xt_d, 0, 8, 512 * i, 512 * (i + 1))

            # ---- phase B: K'^T = (Wk Wq^T)^T x^T over local s ----
            # The first two chains run k-banded in the order the staged
            # critical loads land (k0-3 ~11.5us, k4-5 ~13.3, k6-7 ~15.1), so
            # the PE does real work during the load window instead of longer
            # warmup. Interleaved open accumulation groups across the two
            # ps512 banks are fine (same pattern as attn@V's pso/pss chains).
            psA = [psum.tile([P, 512], f32, tag="ps512", bufs=3, name=f"psA{_m}")
                   for _m in range(2)]
            for kb0, kb1 in ((0, 4), (4, 6), (6, 8)):
                for mm in range(2):
                    for k in range(kb0, kb1):
                        nc.tensor.matmul(psA[mm][:], wq_sb[:, k, mm * P:(mm + 1) * P],
                                         xl_sb[:, k, 0:512],
                                         start=(k == 0), stop=(k == NK - 1))
                if kb1 < NK:
                    # dependency-free fillers into the (dead) warm slot: on a
                    # core whose next k-band lands late (per-core DMA variance
                    # is ~1-2us), these keep the PE active so the >3.4us HAM
                    # re-throttle window never opens. Reuse wps directly -- a
                    # fresh ps512 allocation here would rotate onto a slot
                    # holding an open psA accumulation and its start=True
                    # would clear that bank.
                    for _f in range(2):
                        nc.tensor.matmul(wps[:], warm[:, 0:P], warm[:],
                                         start=True, stop=True)
            for mm in range(2):
                nc.vector.tensor_copy(kt_sb[:, mm, 0:512], psA[mm][:])
            for h in range(2):                     # dout halves
                for j in range(2):                 # local s 512-chunks
                    for mm in range(4):
                        if h == 0 and j == 0 and mm < 2:
                            continue
                        if h == 0 and j == 1 and mm == 0:
                            # same insurance at the j1-load boundary
                            for _f in range(2):
                                nc.tensor.matmul(wps[:], warm[:, 0:P], warm[:],
                                                 start=True, stop=True)
                        m = 4 * h + mm
                        c = 512 * h + mm * P
                        ps = psum.tile([P, 512], f32, tag="ps512", bufs=3)
                        for k in range(NK):
                            nc.tensor.matmul(ps[:], wq_sb[:, k, c:c + P],
                                             xl_sb[:, k, 512 * j:512 * (j + 1)],
                                             start=(k == 0), stop=(k == NK - 1))
                        nc.vector.tensor_copy(kt_sb[:, m, 512 * j:512 * (j + 1)], ps[:])

            # ---- phase D score helper ----
            # scores fold the Q projection into the host-precomputed wkq, so
            # the S^T matmul consumes resident x^T columns.
            def scores_tile(pt, i, lt):
                # the last local tile (lt == 2i+1) is fully masked for the
                # first 256 query columns AND excluded from their attn@V
                # accumulation (nlt), so only its right half is computed
                lo = 256 if lt == 2 * i + 1 else 0
                ps = psum.tile([P, 512 - lo], f32, tag="ps512", bufs=3)
                for m in range(NK):
                    nc.tensor.matmul(ps[:], kt_sb[:, m, lt * P:(lt + 1) * P],
                                     xt_sb[:, m, 512 * i + lo:512 * (i + 1)],
                                     start=(m == 0), stop=(m == NK - 1))
                if lt == 2 * i:
                    nc.vector.tensor_add(ps[:], ps[:], mask_t[:, 0, :])
                elif lt == 2 * i + 1:
                    nc.vector.tensor_add(ps[:], ps[:], mask_t[:, 1, 256:512])
                nc.scalar.activation(pt[:, lt, lo:512], ps[:],
                                     mybir.ActivationFunctionType.Exp,
                                     bias=0.0, scale=SCALE)

            def scores(i):
                pt = pool.tile([P, NK, 512], bf, tag="pt", bufs=2)
                for lt in range(2 * i + 2):
                    scores_tile(pt, i, lt)
                return pt

            # ---- phase C: V over local s ----
            def cgroup(n, j, lt4):
                lt = 4 * j + lt4
                ps = psum.tile([P, 512], f32, tag="ps512", bufs=3)
                for k in range(NK):
                    nc.tensor.matmul(ps[:],
                                     xl_sb[:, k, lt * P:(lt + 1) * P],
                                     wv_sb[:, k, 512 * n:512 * (n + 1)],
                                     start=(k == 0), stop=(k == NK - 1))
                nc.vector.tensor_copy(v_sb[:, lt, 512 * n:512 * (n + 1)], ps[:])

            # chunk 0's two score tiles are interleaved into phase C so each
            # serial mask-add + exp chain hides under a C group (exposed 0.7us
            # on the PE at any phase boundary otherwise), with two C groups
            # between the score tile and the next reuse of its PSUM slot.
            for lt4 in range(4):
                cgroup(0, 0, lt4)
            pt0 = pool.tile([P, NK, 512], bf, tag="pt", bufs=2)
            scores_tile(pt0, 0, 0)
            cgroup(0, 1, 0)
            cgroup(0, 1, 1)
            scores_tile(pt0, 0, 1)
            cgroup(0, 1, 2)
            cgroup(0, 1, 3)
            for n, j in ((1, 0), (1, 1)):
                for lt4 in range(4):
                    cgroup(n, j, lt4)

            # ---- phase D: per query chunk ----
            for i in range(QC):
                pt = pt0 if i == 0 else scores(i)
                qb_order = [3, 2, 1, 0] if i == QC - 1 else [0, 1, 2, 3]
                for qb in qb_order:
                    nlt = 2 * i + 1 if qb < 2 else 2 * i + 2
                    pso = psum.tile([P, D], f32, tag="psO", bufs=2)
                    pss = psum.tile([P, 2], f32, tag="psS", bufs=1)
                    for t_ in range(nlt):
                        lhs = pt[:, t_, qb * P:(qb + 1) * P]
                        st, sp = (t_ == 0), (t_ == nlt - 1)
                        nc.tensor.matmul(pso[:, 0:512], lhs, v_sb[:, t_, 0:512],
                                         start=st, stop=sp)
                        nc.tensor.matmul(pso[:, 512:1024], lhs, v_sb[:, t_, 512:1024],
                                         start=st, stop=sp)
                        nc.tensor.matmul(pss[:], lhs, ones_t[:], start=st, stop=sp)
                    # 4 bufs: output-DMA completion can take ~3-6us under
                    # cross-core HBM contention; 2 bufs convoys the casts
                    # (cast qb_n+2 gated on dma qb_n) and stalls psO recycling
                    osb = pool.tile([P, D + 1], bf, tag="osb", bufs=4)
                    r0 = 512 * i + qb * P
                    # output triggers go on SP only: on the ACT ring they
                    # queue ahead of the next chunk's Exp activations (FIFO)
                    # and stall PSUM recycling for ~1us per chunk boundary
                    if i < QC - 1:
                        # 262KB bf16 per block: one trigger mid-kernel
                        nc.vector.tensor_copy(osb[:, 0:D], pso[:])
                        nc.vector.tensor_copy(osb[:, D:D + 1], pss[:, 0:1])
                        nc.sync.dma_start(part_d[r0:r0 + P, :], osb[:])
                    else:
                        # tail: pipeline copy quarters with their DMAs so the
                        # drain starts ~400ns after the last matmul
                        nc.vector.tensor_copy(osb[:, D:D + 1], pss[:, 0:1])
                        # the tail is SP-ISSUE-bound: exec ends at the last
                        # output DMA's completion, and each trigger costs
                        # ~0.6us of serial sequencer time. In the final chunk
                        # ACT is free (all its exps precede attn@V), so
                        # alternate rings with 2 pieces per block: the issue
                        # train shrinks from ~16 SP triggers (~10us past the
                        # last matmul) to 4 per ring. (A 4-piece final block
                        # measured no better: the extra copies/triggers cost
                        # what the smaller final transfer saves.)
                        pieces = ((0, 512), (512, D + 1))
                        for piece, (c_lo, c_hi) in enumerate(pieces):
                            nc.vector.tensor_copy(osb[:, c_lo:min(c_hi, D)],
                                                  pso[:, c_lo:min(c_hi, D)])
                            e = nc.sync if piece % 2 == 0 else nc.scalar
                            e.dma_start(part_d[r0:r0 + P, c_lo:c_hi],
                                        osb[:, c_lo:c_hi])

    nc.finalize()
    return nc


def _get_program():
    global _prog
    if _prog is None:
        _prog = _build_program()
    return _prog


def kernel(x, Wq, Wk, Wv):
    import ml_dtypes
    from concourse.bass_utils import run_bass_kernel_spmd

    bf16 = ml_dtypes.bfloat16
    x = np.asarray(x, dtype=np.float32)
    Wq = np.ascontiguousarray(np.asarray(Wq, dtype=np.float32))
    Wk = np.ascontiguousarray(np.asarray(Wk, dtype=np.float32))
    Wv = np.ascontiguousarray(np.asarray(Wv, dtype=np.float32))

    ones = np.ones((P, 2), dtype=bf16)
    # scores = x (Wq Wk^T) x^T: fold the two projection matrices on the host.
    # The device tensor plays the old Wk role: lhsT[b, a] = (Wk Wq^T)[b, a].
    Wkq = np.ascontiguousarray(
        (Wk.astype(np.float64) @ Wq.T.astype(np.float64)).astype(np.float32)
    ).astype(bf16)
    Wv_b = np.ascontiguousarray(Wv).astype(bf16)
    sr = np.arange(P)[:, None]
    qr = np.arange(512)[None, :]
    masks = {}
    for p in (0, 1):
        m0 = np.where(128 * p + sr > qr, NEG, 0.0).astype(bf16)
        m1 = np.where(128 * (2 + p) + sr > qr, NEG, 0.0).astype(bf16)
        masks[p] = np.stack([m0, m1])

    in_maps = []
    for c in range(8):
        b, p = c // 2, c % 2
        xt = np.ascontiguousarray(x[b].T)                     # [D, T]
        xtv = xt.reshape(D, T // P, P)
        xtl = np.ascontiguousarray(
            xtv[:, p::2, :].reshape(D, T // 2)).astype(bf16)  # local s cols
        in_maps.append({
            "xt": xt.astype(bf16), "xtl": xtl,
            "wkq": Wkq, "wv": Wv_b,
            "masks": masks[p], "ones": ones,
        })

    global _last_in_maps
    _last_in_maps = in_maps
    nc = _get_program()
    res = run_bass_kernel_spmd(nc, in_maps, list(range(8)))

    out = np.empty((B, T, D), dtype=np.float32)
    for b in range(B):
        p0 = res.results[2 * b]["part"].astype(np.float32)
        p1 = res.results[2 * b + 1]["part"].astype(np.float32)
        O = p0[:, :D] + p1[:, :D]
        d = p0[:, D] + p1[:, D]
        out[b] = O / d[:, None]
    return out
